# revision 1
# baseline (speedup 1.0000x reference)
"""DeepseekV2 MoE layer on 8 Trainium2 NeuronCores.

Strategy (expert-parallel, per the sharding hint):
  - Router gate + grouped top-k computed on host (0.03% of module FLOPs);
    it determines the dispatch, which IS the input sharding.
  - 16 routed experts paired big-count-with-small-count onto 8 cores
    (2 experts per core, token lists gathered host-side, padded to a
    shared per-slot capacity so all cores run one SPMD program).
  - Shared-expert MLP is data-parallel over tokens: each core runs
    T/8 = 512 tokens through the full shared MLP.
  - All matmuls in bf16 (fp32 PE matmul is 2x slower), f32 PSUM
    accumulation, f32 outputs.
  - Device computes outputs token-on-free-dim (transposed); host
    transposes/combines during unshard.
"""

import sys

sys.path.insert(0, "/opt/trn_rl_repo")

import copy

import ml_dtypes
import numpy as np

import concourse.bass as bass
import concourse.mybir as mybir
import concourse.tile as tile
from concourse.bass_utils import run_bass_kernel_spmd

DT = mybir.dt
BF16 = ml_dtypes.bfloat16

T, D, E, I = 4096, 2048, 16, 1024
TOP_K, N_GROUP, TOPK_GROUP = 4, 4, 2
ROUTED_SCALE = 2.5
SHARED_I = 2048
N_CORES = 8
P = 128
NCHUNK = 512  # token chunk (matmul moving free dim)


# ---------------------------------------------------------------- wait split
def _split_excess_waits(nc, limit=1):
    """This walrus build rejects >1 sync-wait command per instruction.
    Move excess waits onto fresh same-engine NOPs inserted just before."""
    template = bass.Bass(target_bir_lowering=False).sync.nop(nofuse=True).ins
    ctr = 0
    for bb in nc.main_func.blocks:
        out = []
        changed = False
        for ins in bb.instructions:
            si = ins.sync_info
            if si is not None and si.on_wait and len(si.on_wait) > limit:
                waits = list(si.on_wait)
                for w in waits[:-limit]:
                    ctr += 1
                    nop = copy.deepcopy(template)
                    nop.name = f"I-wsplit-{ctr}"
                    nop.engine = ins.engine
                    nop.bass_nofuse = True
                    nop.sync_info = mybir.SyncInfo(on_wait=[w], on_update=[])
                    nc.register_instruction(nop, overwrite=True)
                    out.append(nop)
                ins.sync_info = mybir.SyncInfo(
                    on_wait=waits[-limit:], on_update=list(si.on_update)
                )
                changed = True
            out.append(ins)
        if changed:
            bb.instructions = out
    return ctr


# ---------------------------------------------------------------- routing
def _gate_logits(x, gate_w):
    # Match the reference's jax-f32 CPU matmul as closely as possible.
    try:
        import jax
        import jax.numpy as jnp

        cpu = jax.devices("cpu")[0]
        with jax.default_device(cpu):
            return np.asarray(jnp.matmul(jnp.asarray(x), jnp.asarray(gate_w)))
    except Exception:
        return (x @ gate_w).astype(np.float32)


def _route(x, gate_w, e_bias):
    logits = _gate_logits(x, gate_w)  # [T, E] f32
    scores = (1.0 / (1.0 + np.exp(-logits))).astype(np.float32)
    sfc = scores + e_bias[None, :]
    grp = sfc.reshape(T, N_GROUP, E // N_GROUP)
    group_scores = np.sort(grp, axis=-1)[:, :, -2:].sum(-1)  # [T, G]
    group_idx = np.argsort(-group_scores, axis=-1, kind="stable")[:, :TOPK_GROUP]
    group_mask = np.zeros((T, N_GROUP), bool)
    group_mask[np.arange(T)[:, None], group_idx] = True
    expert_mask = np.repeat(group_mask, E // N_GROUP, axis=1)
    masked = np.where(expert_mask, sfc, -np.inf)
    topk_idx = np.argsort(-masked, axis=-1, kind="stable")[:, :TOP_K]  # [T, 4]
    topk_w = np.take_along_axis(scores, topk_idx, axis=1)
    topk_w = topk_w / topk_w.sum(axis=1, keepdims=True)
    return topk_idx.astype(np.int64), topk_w.astype(np.float32)


# ---------------------------------------------------------------- program
_PROGRAM_CACHE = {}


def _emit_expert(nc, tc, pools, xt_h, w1_h, w2_h, wr_h, y_h, C, twoI, apply_wr):
    n_d = D // P  # 16 contraction chunks over D
    n_i = twoI // P  # gate_up output chunks
    n_h = n_i // 2  # h chunks (= I/128)
    chunks = [(o, min(NCHUNK, C - o)) for o in range(0, C, NCHUNK)]

    (xt_pool, w1_pool, w2_pool, g_pool, h_pool, y_pool, wr_pool, sg_pool,
     ps_gu, ps_dn) = pools

    # whole-expert X^T resident tile: [p, k(d-chunk), tok]; split the load
    # per d-chunk so the first matmuls start as soon as chunk 0 lands
    xt_t = xt_pool.tile([P, n_d, C], DT.bfloat16, name="xt")
    src = xt_h[:, :].rearrange("(k p) t -> p k t", p=P)
    for d in range(n_d):
        nc.sync.dma_start(xt_t[:, d, :], src[:, d, :])

    wr_t = None
    if apply_wr:
        wr_t = wr_pool.tile([P, C], DT.float32, name="wr")
        nc.sync.dma_start(wr_t[:], wr_h[:, :])

    g_tiles = {}
    h_tiles = {}
    for i in range(n_i):
        w1s = w1_pool.tile([P, n_d, P], DT.bfloat16, name="w1s")
        nc.sync.dma_start(w1s[:], w1_h[i])
        for ci, (off, sz) in enumerate(chunks):
            ps = ps_gu.tile([P, NCHUNK], DT.float32, name="psg")
            for d in range(n_d):
                nc.tensor.matmul(
                    ps[:, :sz],
                    w1s[:, d, :],
                    xt_t[:, d, off : off + sz],
                    start=(d == 0),
                    stop=(d == n_d - 1),
                )
            if i < n_h:
                sg = sg_pool.tile([P, NCHUNK], DT.float32, name="sg")
                nc.scalar.activation(
                    sg[:, :sz], ps[:, :sz], mybir.ActivationFunctionType.Sigmoid
                )
                gt = g_pool.tile([P, NCHUNK], DT.float32, name="gt")
                nc.vector.tensor_mul(gt[:, :sz], ps[:, :sz], sg[:, :sz])
                g_tiles[(i, ci)] = gt
            else:
                ht = h_pool.tile([P, NCHUNK], DT.bfloat16, name="ht")
                nc.vector.tensor_mul(
                    ht[:, :sz], ps[:, :sz], g_tiles[(i - n_h, ci)][:, :sz]
                )
                h_tiles[(i - n_h, ci)] = ht

    for d2 in range(D // P):
        w2s = w2_pool.tile([P, n_h, P], DT.bfloat16, name="w2s")
        nc.sync.dma_start(w2s[:], w2_h[d2])
        for ci, (off, sz) in enumerate(chunks):
            ps = ps_dn.tile([P, NCHUNK], DT.float32, name="psd")
            for hh in range(n_h):
                nc.tensor.matmul(
                    ps[:, :sz],
                    w2s[:, hh, :],
                    h_tiles[(hh, ci)][:, :sz],
                    start=(hh == 0),
                    stop=(hh == n_h - 1),
                )
            ys = y_pool.tile([P, NCHUNK], DT.float32, name="ys")
            if apply_wr:
                nc.vector.tensor_mul(ys[:, :sz], ps[:, :sz], wr_t[:, off : off + sz])
            else:
                nc.scalar.copy(ys[:, :sz], ps[:, :sz])
            nc.sync.dma_start(y_h[d2 * P : (d2 + 1) * P, off : off + sz], ys[:, :sz])


def _build_program(C1, C2):
    key = (C1, C2)
    if key in _PROGRAM_CACHE:
        return _PROGRAM_CACHE[key]

    nc = bass.Bass(target_bir_lowering=False)
    TS = T // N_CORES  # shared tokens per core

    xt1 = nc.dram_tensor("xt1", [D, C1], DT.bfloat16, kind="ExternalInput")
    xt2 = nc.dram_tensor("xt2", [D, C2], DT.bfloat16, kind="ExternalInput")
    xts = nc.dram_tensor("xts", [D, TS], DT.bfloat16, kind="ExternalInput")
    w1a = nc.dram_tensor("w1a", [2 * I // P, P, D // P, P], DT.bfloat16, kind="ExternalInput")
    w2a = nc.dram_tensor("w2a", [D // P, P, I // P, P], DT.bfloat16, kind="ExternalInput")
    w1b = nc.dram_tensor("w1b", [2 * I // P, P, D // P, P], DT.bfloat16, kind="ExternalInput")
    w2b = nc.dram_tensor("w2b", [D // P, P, I // P, P], DT.bfloat16, kind="ExternalInput")
    ws1 = nc.dram_tensor("ws1", [2 * SHARED_I // P, P, D // P, P], DT.bfloat16, kind="ExternalInput")
    ws2 = nc.dram_tensor("ws2", [D // P, P, SHARED_I // P, P], DT.bfloat16, kind="ExternalInput")
    wr1 = nc.dram_tensor("wr1", [P, C1], DT.float32, kind="ExternalInput")
    wr2 = nc.dram_tensor("wr2", [P, C2], DT.float32, kind="ExternalInput")
    y1 = nc.dram_tensor("y1", [D, C1], DT.float32, kind="ExternalOutput")
    y2 = nc.dram_tensor("y2", [D, C2], DT.float32, kind="ExternalOutput")
    ys = nc.dram_tensor("ys", [D, TS], DT.float32, kind="ExternalOutput")

    with tile.TileContext(nc) as tc:
        with (
            tc.tile_pool(name="xt", bufs=1) as xt_pool,
            tc.tile_pool(name="w1p", bufs=3) as w1_pool,
            tc.tile_pool(name="w2p", bufs=3) as w2_pool,
            tc.tile_pool(name="gp", bufs=24) as g_pool,
            tc.tile_pool(name="hp", bufs=24) as h_pool,
            tc.tile_pool(name="yp", bufs=4) as y_pool,
            tc.tile_pool(name="wrp", bufs=2) as wr_pool,
            tc.tile_pool(name="sgp", bufs=3) as sg_pool,
            tc.tile_pool(name="psgu", bufs=4, space="PSUM") as ps_gu,
            tc.tile_pool(name="psdn", bufs=4, space="PSUM") as ps_dn,
        ):
            pools = (xt_pool, w1_pool, w2_pool, g_pool, h_pool, y_pool, wr_pool,
                     sg_pool, ps_gu, ps_dn)
            _emit_expert(nc, tc, pools, xt1, w1a, w2a, wr1, y1, C1, 2 * I, True)
            _emit_expert(nc, tc, pools, xt2, w1b, w2b, wr2, y2, C2, 2 * I, True)
            _emit_expert(nc, tc, pools, xts, ws1, ws2, None, ys, T // N_CORES, 2 * SHARED_I, False)

    _split_excess_waits(nc, limit=1)
    _PROGRAM_CACHE[key] = nc
    return nc


# ---------------------------------------------------------------- packing
def _pack_w1(w, twoI):  # w [D, twoI] f32 -> [twoI/P, P, D/P, P] bf16
    n_d, n_i = D // P, twoI // P
    return np.ascontiguousarray(
        w.astype(BF16).reshape(n_d, P, n_i, P).transpose(2, 1, 0, 3)
    )


def _pack_w2(w, I_):  # w [I_, D] f32 -> [D/P, P, I_/P, P] bf16
    n_h, n_d2 = I_ // P, D // P
    return np.ascontiguousarray(
        w.astype(BF16).reshape(n_h, P, n_d2, P).transpose(2, 1, 0, 3)
    )


def _cap(n):
    # exact capacity (matmul free dim handles any size <= 512 per chunk);
    # keep a small floor so shapes stay sane for degenerate routings
    return max(P, int(n))


# ---------------------------------------------------------------- kernel
def _prepare(hidden_states, gate_w, e_bias, w_gate_up, w_down, ws_gate_up, ws_down):
    x = np.asarray(hidden_states, dtype=np.float32)
    topk_idx, topk_w = _route(x, np.asarray(gate_w), np.asarray(e_bias))

    # dispatch: token lists per expert, sorted-stable by expert id
    flat_e = topk_idx.ravel()
    order = np.argsort(flat_e, kind="stable")
    pair_tok = order // TOP_K
    pair_w = (topk_w.ravel()[order] * ROUTED_SCALE).astype(np.float32)
    counts = np.bincount(flat_e, minlength=E)
    starts = np.zeros(E + 1, np.int64)
    np.cumsum(counts, out=starts[1:])

    # expert -> core assignment: pair largest with smallest
    by_count = np.argsort(-counts, kind="stable")
    slotA = by_count[:N_CORES]
    slotB = by_count[E - 1 : N_CORES - 1 : -1]  # reversed smallest half
    C1 = _cap(counts[slotA].max())
    C2 = _cap(counts[slotB].max())

    nc = _build_program(C1, C2)

    x_bf = x.astype(BF16)
    xT_bf = np.ascontiguousarray(x_bf.T)  # [D, T]

    ws1_p = _pack_w1(np.asarray(ws_gate_up), 2 * SHARED_I)
    ws2_p = _pack_w2(np.asarray(ws_down), SHARED_I)
    w_gate_up = np.asarray(w_gate_up)
    w_down = np.asarray(w_down)

    TS = T // N_CORES
    in_maps = []
    core_info = []
    for c in range(N_CORES):
        eA, eB = int(slotA[c]), int(slotB[c])
        m = {}
        info = []
        for slot, e_id, C, xt_name, wr_name in (
            (0, eA, C1, "xt1", "wr1"),
            (1, eB, C2, "xt2", "wr2"),
        ):
            idx = pair_tok[starts[e_id] : starts[e_id + 1]]
            w = pair_w[starts[e_id] : starts[e_id + 1]]
            n_e = len(idx)
            idx_pad = np.zeros(C, np.int64)
            idx_pad[:n_e] = idx
            w_pad = np.zeros(C, np.float32)
            w_pad[:n_e] = w
            m[xt_name] = xT_bf[:, idx_pad]
            m[wr_name] = np.ascontiguousarray(
                np.broadcast_to(w_pad, (P, C))
            )
            info.append((idx, n_e))
        m["xts"] = np.ascontiguousarray(xT_bf[:, c * TS : (c + 1) * TS])
        m["w1a"] = _pack_w1(w_gate_up[eA], 2 * I)
        m["w2a"] = _pack_w2(w_down[eA], I)
        m["w1b"] = _pack_w1(w_gate_up[eB], 2 * I)
        m["w2b"] = _pack_w2(w_down[eB], I)
        m["ws1"] = ws1_p
        m["ws2"] = ws2_p
        in_maps.append(m)
        core_info.append(info)
    return nc, in_maps, core_info


def _combine(res_results, core_info):
    TS = T // N_CORES
    out = np.zeros((T, D), np.float32)
    for c in range(N_CORES):
        (idxA, nA), (idxB, nB) = core_info[c]
        out[idxA] += res_results[c]["y1"][:, :nA].T
        out[idxB] += res_results[c]["y2"][:, :nB].T
        out[c * TS : (c + 1) * TS] += res_results[c]["ys"].T
    return out


def kernel(hidden_states, gate_w, e_bias, w_gate_up, w_down, ws_gate_up, ws_down):
    nc, in_maps, core_info = _prepare(
        hidden_states, gate_w, e_bias, w_gate_up, w_down, ws_gate_up, ws_down
    )
    res = run_bass_kernel_spmd(nc, in_maps, list(range(N_CORES)))
    return _combine(res.results, core_info)



# revision 14
# speedup vs baseline: 1.3636x; 1.3636x over previous
"""DeepseekV2 MoE layer on 8 Trainium2 NeuronCores.

Strategy (expert-parallel, per the sharding hint):
  - Router gate + grouped top-k computed on host (0.03% of module FLOPs);
    it determines the dispatch, which IS the input sharding.
  - 16 routed experts paired big-count-with-small-count onto 8 cores
    (2 experts per core, token lists gathered host-side, padded to a
    shared per-slot capacity so all cores run one SPMD program).
  - Shared-expert MLP is data-parallel over tokens: each core runs
    T/8 = 512 tokens through the full shared MLP.
  - All matmuls run as fp8-e4m3 DoubleRow (2 K-blocks per instruction at
    0.5 cycles/row = 4x bf16 PE throughput), with 3-term error
    compensation per matmul:  X@W ~= Xh@Wh + Xh@Wl + Xl@Wh  where
    Xh/Wh are e4m3 quantizations and Xl/Wl the e4m3-quantized residuals.
    Net PE cost is 0.75x bf16 for ~2e-3 rel error (vs 4e-3 for bf16).
  - Quantization scales are global (shared across experts/cores) so all
    dequant factors are compile-time immediates in the single SPMD
    program; per-token routing weights fold the down-proj dequant.
  - x and all weights are quantized host-side; only the MLP intermediate
    h is quantized (hi+lo) on device.
"""

import sys

sys.path.insert(0, "/opt/trn_rl_repo")

import copy

import ml_dtypes
import numpy as np

import concourse.bass as bass
import concourse.mybir as mybir
import concourse.tile as tile
from concourse.bass_utils import run_bass_kernel_spmd

DT = mybir.dt
E4NP = ml_dtypes.float8_e4m3  # TRN float8e4: max-normal 240
AF = mybir.ActivationFunctionType
DR = mybir.MatmulPerfMode.DoubleRow
MUL = mybir.AluOpType.mult

T, D, E, I = 4096, 2048, 16, 1024
TOP_K, N_GROUP, TOPK_GROUP = 4, 4, 2
ROUTED_SCALE = 2.5
SHARED_I = 2048
N_CORES = 8
P = 128
NCHUNK = 512  # token chunk (DoubleRow moving free dim = 2*NCHUNK fp8 = 512 bus elems)
S_H = 14.0    # fixed h quant scale (|h| stays < ~11; range to 240/14 = 17)


# ---------------------------------------------------------------- wait split
def _split_excess_waits(nc, limit=1):
    """This walrus build rejects >1 sync-wait command per instruction.
    Move excess waits onto fresh same-engine NOPs inserted just before."""
    template = bass.Bass(target_bir_lowering=False).sync.nop(nofuse=True).ins
    ctr = 0
    for bb in nc.main_func.blocks:
        out = []
        changed = False
        for ins in bb.instructions:
            si = ins.sync_info
            if si is not None and si.on_wait and len(si.on_wait) > limit:
                waits = list(si.on_wait)
                for w in waits[:-limit]:
                    ctr += 1
                    nop = copy.deepcopy(template)
                    nop.name = f"I-wsplit-{ctr}"
                    nop.engine = ins.engine
                    nop.bass_nofuse = True
                    nop.sync_info = mybir.SyncInfo(on_wait=[w], on_update=[])
                    nc.register_instruction(nop, overwrite=True)
                    out.append(nop)
                ins.sync_info = mybir.SyncInfo(
                    on_wait=waits[-limit:], on_update=list(si.on_update)
                )
                changed = True
            out.append(ins)
        if changed:
            bb.instructions = out
    return ctr


# ---------------------------------------------------------------- routing
def _gate_logits(x, gate_w):
    # Match the reference's jax-f32 CPU matmul as closely as possible.
    try:
        import jax
        import jax.numpy as jnp

        cpu = jax.devices("cpu")[0]
        with jax.default_device(cpu):
            return np.asarray(jnp.matmul(jnp.asarray(x), jnp.asarray(gate_w)))
    except Exception:
        return (x @ gate_w).astype(np.float32)


def _route(x, gate_w, e_bias):
    logits = _gate_logits(x, gate_w)  # [T, E] f32
    scores = (1.0 / (1.0 + np.exp(-logits))).astype(np.float32)
    sfc = scores + e_bias[None, :]
    grp = sfc.reshape(T, N_GROUP, E // N_GROUP)
    group_scores = np.sort(grp, axis=-1)[:, :, -2:].sum(-1)  # [T, G]
    group_idx = np.argsort(-group_scores, axis=-1, kind="stable")[:, :TOPK_GROUP]
    group_mask = np.zeros((T, N_GROUP), bool)
    group_mask[np.arange(T)[:, None], group_idx] = True
    expert_mask = np.repeat(group_mask, E // N_GROUP, axis=1)
    masked = np.where(expert_mask, sfc, -np.inf)
    topk_idx = np.argsort(-masked, axis=-1, kind="stable")[:, :TOP_K]  # [T, 4]
    topk_w = np.take_along_axis(scores, topk_idx, axis=1)
    topk_w = topk_w / topk_w.sum(axis=1, keepdims=True)
    return topk_idx.astype(np.int64), topk_w.astype(np.float32)


# ---------------------------------------------------------------- quantize
def _q8(a, scale):
    """e4m3-quantize a*scale (returns fp8 array); clip to TRN max 240."""
    return np.clip(a * np.float32(scale), -240.0, 240.0).astype(E4NP)


def _hilo(a, scale):
    hi = _q8(a, scale)
    lo = _q8(a - hi.astype(np.float32) / np.float32(scale), scale)
    return hi, lo


# ---------------------------------------------------------------- program
_PROGRAM_CACHE = {}


def _load_x(nc, xt_pool, xh_h, xl_h, C):
    """Resident x^T hi/lo tiles [p, kblock, tok], loaded in 2-kblock pieces
    (hi first — the first 2/3 of each psum chain reads only hi)."""
    n_d = D // P
    xh_t = xt_pool.tile([P, n_d, C], DT.float8e4, name="xh")
    xl_t = xt_pool.tile([P, n_d, C], DT.float8e4, name="xl")
    src_h = xh_h[:, :].rearrange("(k p) t -> p k t", p=P)
    src_l = xl_h[:, :].rearrange("(k p) t -> p k t", p=P)
    for d in range(0, n_d, 2):
        nc.sync.dma_start(xh_t[:, d : d + 2, :], src_h[:, d : d + 2, :])
    for d in range(0, n_d, 2):
        nc.sync.dma_start(xl_t[:, d : d + 2, :], src_l[:, d : d + 2, :])
    return xh_t, xl_t


def _emit_slot(nc, tc, pools, x_t, w1_h, w2_h, wr_h, y_h, C, twoI,
               inv_g, c_u, c_y, w1_pre=None, hooks=None):
    """One expert slot: y = (silu(x@W1g) * x@W1u) @ W2 [* wr].

    x_t: preloaded (xh_t, xl_t) SBUF tiles
    w1_h: [n_i, P, 2, n_d, P] fp8 dram (dim2 = hi/lo)
    w2_h: [n_d2, P, 2, n_hb, P] fp8 dram
    wr_h: [P, C] f32 dram or None; y_h: [D, C] f32 dram out
    w1_pre: pre-issued (w1g, w1u) tiles for u=0 (startup prefetch)
    """
    n_d = D // P          # 16 contraction blocks for gate_up
    n_i = twoI // P       # gate_up output tiles
    n_h = n_i // 2        # h tiles (= I_/128)
    n_hb = n_h            # contraction blocks for down-proj
    n_d2 = D // P         # down-proj output tiles
    chunks = [(o, min(NCHUNK, C - o)) for o in range(0, C, NCHUNK)]

    (xt_pool, w1_pool, w2_pool, sg_pool, hf_pool, h_pool, y_pool, wr_pool,
     ps_g, ps_u, ps_dn) = pools
    xh_t, xl_t = x_t

    wr_t = None
    if wr_h is not None:
        wr_t = wr_pool.tile([P, C], DT.float32, name="wr")
        nc.sync.dma_start(wr_t[:], wr_h[:, :])

    hh_t = h_pool.tile([P, n_h, C], DT.float8e4, name="hh")
    hl_t = h_pool.tile([P, n_h, C], DT.float8e4, name="hl")

    # ---- gate_up, g/u tile pairs interleaved so silu partners are fresh
    for u in range(n_h):
        if hooks and u in hooks:
            hooks[u]()
        if u == 0 and w1_pre is not None:
            w1g, w1u = w1_pre
        else:
            w1g = w1_pool.tile([P, 2, n_d, P], DT.float8e4, name="w1g")
            nc.sync.dma_start(w1g[:], w1_h[u])
            w1u = w1_pool.tile([P, 2, n_d, P], DT.float8e4, name="w1u")
            nc.sync.dma_start(w1u[:], w1_h[n_h + u])
        for off, sz in chunks:
            psg = ps_g.tile([P, NCHUNK], DT.float32, name="psg")
            psu = ps_u.tile([P, NCHUNK], DT.float32, name="psu")
            for ps, wt in ((psg, w1g), (psu, w1u)):
                step = 0
                for wi, xt in ((0, xh_t), (1, xh_t), (0, xl_t)):
                    for kp in range(n_d // 2):
                        nc.tensor.matmul(
                            ps[:, :sz],
                            wt[:, wi, 2 * kp : 2 * kp + 2, :],
                            xt[:, 2 * kp : 2 * kp + 2, off : off + sz],
                            start=(step == 0),
                            stop=(step == 3 * (n_d // 2) - 1),
                            perf_mode=DR,
                        )
                        step += 1
            sgt = sg_pool.tile([P, NCHUNK], DT.float32, name="sgt")
            nc.scalar.activation(sgt[:, :sz], psg[:, :sz], AF.Sigmoid, scale=inv_g)
            gt = sg_pool.tile([P, NCHUNK], DT.float32, name="gt")
            nc.vector.scalar_tensor_tensor(
                gt[:, :sz], psg[:, :sz], inv_g, sgt[:, :sz], op0=MUL, op1=MUL
            )
            hf = hf_pool.tile([P, NCHUNK], DT.float32, name="hf")
            nc.vector.scalar_tensor_tensor(
                hf[:, :sz], psu[:, :sz], c_u, gt[:, :sz], op0=MUL, op1=MUL
            )
            nc.vector.tensor_copy(hh_t[:, u, off : off + sz], hf[:, :sz])
            nc.vector.tensor_sub(
                hl_t[:, u, off : off + sz], hf[:, :sz], hh_t[:, u, off : off + sz]
            )

    # ---- down-proj
    for d2 in range(n_d2):
        w2t = w2_pool.tile([P, 2, n_hb, P], DT.float8e4, name="w2t")
        nc.sync.dma_start(w2t[:], w2_h[d2])
        yt = y_pool.tile([P, C], DT.float32, name="yt")
        for off, sz in chunks:
            ps2 = ps_dn.tile([P, NCHUNK], DT.float32, name="psd")
            step = 0
            for wi, ht in ((0, hh_t), (1, hh_t), (0, hl_t)):
                for hp in range(n_hb // 2):
                    nc.tensor.matmul(
                        ps2[:, :sz],
                        w2t[:, wi, 2 * hp : 2 * hp + 2, :],
                        ht[:, 2 * hp : 2 * hp + 2, off : off + sz],
                        start=(step == 0),
                        stop=(step == 3 * (n_hb // 2) - 1),
                        perf_mode=DR,
                    )
                    step += 1
            if wr_t is not None:
                nc.vector.tensor_mul(
                    yt[:, off : off + sz], ps2[:, :sz], wr_t[:, off : off + sz]
                )
            else:
                nc.scalar.activation(
                    yt[:, off : off + sz], ps2[:, :sz], AF.Copy, scale=c_y
                )
        nc.sync.dma_start(y_h[d2 * P : (d2 + 1) * P, :], yt[:])


def _build_program(C1, C2, inv_g, c_u, c_y):
    key = (C1, C2, round(float(inv_g), 10), round(float(c_u), 10),
           round(float(c_y), 10))
    if key in _PROGRAM_CACHE:
        return _PROGRAM_CACHE[key]

    nc = bass.Bass(target_bir_lowering=False)
    TS = T // N_CORES  # shared tokens per core

    def din(name, shape, dt=DT.float8e4):
        return nc.dram_tensor(name, shape, dt, kind="ExternalInput")

    xt1h = din("xt1h", [D, C1]); xt1l = din("xt1l", [D, C1])
    xt2h = din("xt2h", [D, C2]); xt2l = din("xt2l", [D, C2])
    xtsh = din("xtsh", [D, TS]); xtsl = din("xtsl", [D, TS])
    w1a = din("w1a", [2 * I // P, P, 2, D // P, P])
    w2a = din("w2a", [D // P, P, 2, I // P, P])
    w1b = din("w1b", [2 * I // P, P, 2, D // P, P])
    w2b = din("w2b", [D // P, P, 2, I // P, P])
    ws1 = din("ws1", [2 * SHARED_I // P, P, 2, D // P, P])
    ws2 = din("ws2", [D // P, P, 2, SHARED_I // P, P])
    wr1 = din("wr1", [P, C1], DT.float32)
    wr2 = din("wr2", [P, C2], DT.float32)
    y1 = nc.dram_tensor("y1", [D, C1], DT.float32, kind="ExternalOutput")
    y2 = nc.dram_tensor("y2", [D, C2], DT.float32, kind="ExternalOutput")
    ys = nc.dram_tensor("ys", [D, TS], DT.float32, kind="ExternalOutput")

    with tile.TileContext(nc) as tc:
        with (
            tc.tile_pool(name="xt", bufs=2) as xt_pool,
            tc.tile_pool(name="xts", bufs=1) as xts_pool,
            tc.tile_pool(name="w1p", bufs=2) as w1_pool,
            tc.tile_pool(name="w2p", bufs=3) as w2_pool,
            tc.tile_pool(name="sgp", bufs=3) as sg_pool,
            tc.tile_pool(name="hfp", bufs=3) as hf_pool,
            tc.tile_pool(name="hp", bufs=2) as h_pool,
            tc.tile_pool(name="yp", bufs=3) as y_pool,
            tc.tile_pool(name="wrp", bufs=2) as wr_pool,
            tc.tile_pool(name="psg", bufs=2, space="PSUM") as ps_g,
            tc.tile_pool(name="psu", bufs=3, space="PSUM") as ps_u,
            tc.tile_pool(name="psdn", bufs=3, space="PSUM") as ps_dn,
        ):
            pools = (xt_pool, w1_pool, w2_pool, sg_pool, hf_pool, h_pool,
                     y_pool, wr_pool, ps_g, ps_u, ps_dn)
            # startup: first weight pair of slot A ahead of the bulk X loads
            n_h1 = I // P
            w1g0 = w1_pool.tile([P, 2, D // P, P], DT.float8e4, name="w1g")
            nc.sync.dma_start(w1g0[:], w1a[0])
            w1u0 = w1_pool.tile([P, 2, D // P, P], DT.float8e4, name="w1u")
            nc.sync.dma_start(w1u0[:], w1a[n_h1])
            xA = _load_x(nc, xt_pool, xt1h, xt1l, C1)
            xB = [None]
            xS = [None]
            _emit_slot(nc, tc, pools, xA, w1a, w2a, wr1, y1, C1,
                       2 * I, inv_g, c_u, c_y, w1_pre=(w1g0, w1u0),
                       hooks={4: lambda: xB.__setitem__(
                           0, _load_x(nc, xt_pool, xt2h, xt2l, C2))})
            _emit_slot(nc, tc, pools, xB[0], w1b, w2b, wr2, y2, C2,
                       2 * I, inv_g, c_u, c_y,
                       hooks={2: lambda: xS.__setitem__(
                           0, _load_x(nc, xts_pool, xtsh, xtsl, TS))})
            _emit_slot(nc, tc, pools, xS[0], ws1, ws2, None, ys, TS,
                       2 * SHARED_I, inv_g, c_u, c_y)

    _split_excess_waits(nc, limit=1)
    _PROGRAM_CACHE[key] = nc
    return nc


# ---------------------------------------------------------------- packing
def _pack_w1(w, twoI, s_w):
    """w [D, twoI] f32 -> [twoI/P, P, 2, D/P, P] fp8 (dim2 = hi/lo)."""
    n_d, n_i = D // P, twoI // P
    hi, lo = _hilo(w.astype(np.float32), s_w)

    def pk(a):
        return a.reshape(n_d, P, n_i, P).transpose(2, 1, 0, 3)

    return np.ascontiguousarray(np.stack([pk(hi), pk(lo)], axis=2))


def _pack_w2(w, I_, s_w):
    """w [I_, D] f32 -> [D/P, P, 2, I_/P, P] fp8 (dim2 = hi/lo)."""
    n_h, n_d2 = I_ // P, D // P
    hi, lo = _hilo(w.astype(np.float32), s_w)

    def pk(a):
        return a.reshape(n_h, P, n_d2, P).transpose(2, 1, 0, 3)

    return np.ascontiguousarray(np.stack([pk(hi), pk(lo)], axis=2))


def _cap(n):
    return max(P, int(n))


# ---------------------------------------------------------------- kernel
def _prepare(hidden_states, gate_w, e_bias, w_gate_up, w_down, ws_gate_up, ws_down):
    x = np.asarray(hidden_states, dtype=np.float32)
    topk_idx, topk_w = _route(x, np.asarray(gate_w), np.asarray(e_bias))

    # dispatch: token lists per expert, sorted-stable by expert id
    flat_e = topk_idx.ravel()
    order = np.argsort(flat_e, kind="stable")
    pair_tok = order // TOP_K
    counts = np.bincount(flat_e, minlength=E)
    starts = np.zeros(E + 1, np.int64)
    np.cumsum(counts, out=starts[1:])

    # expert -> core assignment: pair largest with smallest
    by_count = np.argsort(-counts, kind="stable")
    slotA = by_count[:N_CORES]
    slotB = by_count[E - 1 : N_CORES - 1 : -1]  # reversed smallest half
    C1 = _cap(counts[slotA].max())
    C2 = _cap(counts[slotB].max())

    # global quantization scales -> SPMD-safe immediate dequant factors
    w_gate_up = np.asarray(w_gate_up, dtype=np.float32)
    w_down = np.asarray(w_down, dtype=np.float32)
    ws_gate_up = np.asarray(ws_gate_up, dtype=np.float32)
    ws_down = np.asarray(ws_down, dtype=np.float32)
    s_x = 224.0 / max(float(np.abs(x).max()), 1e-30)
    s_w1 = 224.0 / max(float(np.abs(w_gate_up).max()),
                       float(np.abs(ws_gate_up).max()), 1e-30)
    s_w2 = 224.0 / max(float(np.abs(w_down).max()),
                       float(np.abs(ws_down).max()), 1e-30)
    inv_g = 1.0 / (s_x * s_w1)          # dequant of gate_up psum
    c_u = S_H / (s_x * s_w1)            # dequant of u * h-quant scale
    c_y = 1.0 / (S_H * s_w2)            # dequant of down psum (shared)
    pair_w = (topk_w.ravel()[order] * np.float32(ROUTED_SCALE * c_y)).astype(
        np.float32
    )

    nc = _build_program(C1, C2, inv_g, c_u, c_y)

    xT = np.ascontiguousarray(x.T)  # [D, T]
    xTh, xTl = _hilo(xT, s_x)

    ws1_p = _pack_w1(ws_gate_up, 2 * SHARED_I, s_w1)
    ws2_p = _pack_w2(ws_down, SHARED_I, s_w2)

    TS = T // N_CORES
    in_maps = []
    core_info = []
    for c in range(N_CORES):
        eA, eB = int(slotA[c]), int(slotB[c])
        m = {}
        info = []
        for e_id, C, xh_name, xl_name, wr_name in (
            (eA, C1, "xt1h", "xt1l", "wr1"),
            (eB, C2, "xt2h", "xt2l", "wr2"),
        ):
            idx = pair_tok[starts[e_id] : starts[e_id + 1]]
            w = pair_w[starts[e_id] : starts[e_id + 1]]
            n_e = len(idx)
            idx_pad = np.zeros(C, np.int64)
            idx_pad[:n_e] = idx
            w_pad = np.zeros(C, np.float32)
            w_pad[:n_e] = w
            m[xh_name] = xTh[:, idx_pad]
            m[xl_name] = xTl[:, idx_pad]
            m[wr_name] = np.ascontiguousarray(np.broadcast_to(w_pad, (P, C)))
            info.append((idx, n_e))
        m["xtsh"] = np.ascontiguousarray(xTh[:, c * TS : (c + 1) * TS])
        m["xtsl"] = np.ascontiguousarray(xTl[:, c * TS : (c + 1) * TS])
        m["w1a"] = _pack_w1(w_gate_up[eA], 2 * I, s_w1)
        m["w2a"] = _pack_w2(w_down[eA], I, s_w2)
        m["w1b"] = _pack_w1(w_gate_up[eB], 2 * I, s_w1)
        m["w2b"] = _pack_w2(w_down[eB], I, s_w2)
        m["ws1"] = ws1_p
        m["ws2"] = ws2_p
        in_maps.append(m)
        core_info.append(info)
    return nc, in_maps, core_info


def _combine(res_results, core_info):
    TS = T // N_CORES
    out = np.zeros((T, D), np.float32)
    for c in range(N_CORES):
        (idxA, nA), (idxB, nB) = core_info[c]
        out[idxA] += res_results[c]["y1"][:, :nA].T
        out[idxB] += res_results[c]["y2"][:, :nB].T
        out[c * TS : (c + 1) * TS] += res_results[c]["ys"].T
    return out


def kernel(hidden_states, gate_w, e_bias, w_gate_up, w_down, ws_gate_up, ws_down):
    nc, in_maps, core_info = _prepare(
        hidden_states, gate_w, e_bias, w_gate_up, w_down, ws_gate_up, ws_down
    )
    res = run_bass_kernel_spmd(nc, in_maps, list(range(N_CORES)))
    return _combine(res.results, core_info)


# revision 28
# speedup vs baseline: 1.4131x; 1.0363x over previous
"""DeepseekV2 MoE layer on 8 Trainium2 NeuronCores.

Strategy (expert-parallel, per the sharding hint):
  - Router gate + grouped top-k computed on host (0.03% of module FLOPs);
    it determines the dispatch, which IS the input sharding.
  - 16 routed experts paired big-count-with-small-count onto 8 cores
    (2 experts per core, token lists gathered host-side, padded to a
    shared per-slot capacity so all cores run one SPMD program).
  - Shared-expert MLP is data-parallel over tokens: each core runs
    T/8 = 512 tokens through the full shared MLP.
  - All matmuls run as fp8-e4m3 DoubleRow (2 K-blocks per instruction at
    0.5 cycles/row = 4x bf16 PE throughput), with 3-term error
    compensation per matmul:  X@W ~= Xh@Wh + Xh@Wl + Xl@Wh  where
    Xh/Wh are e4m3 quantizations and Xl/Wl the e4m3-quantized residuals.
    Net PE cost is 0.75x bf16 for ~2e-3 rel error (vs 4e-3 for bf16).
  - Quantization scales are global (shared across experts/cores) so all
    dequant factors are compile-time immediates in the single SPMD
    program; per-token routing weights fold the down-proj dequant.
  - x and all weights are quantized host-side; only the MLP intermediate
    h is quantized (hi+lo) on device.
"""

import sys

sys.path.insert(0, "/opt/trn_rl_repo")

import copy

import ml_dtypes
import numpy as np

import concourse.bass as bass
import concourse.mybir as mybir
import concourse.tile as tile
from concourse.bass_utils import run_bass_kernel_spmd

DT = mybir.dt
E4NP = ml_dtypes.float8_e4m3  # TRN float8e4: max-normal 240
AF = mybir.ActivationFunctionType
DR = mybir.MatmulPerfMode.DoubleRow
MUL = mybir.AluOpType.mult

T, D, E, I = 4096, 2048, 16, 1024
TOP_K, N_GROUP, TOPK_GROUP = 4, 4, 2
ROUTED_SCALE = 2.5
SHARED_I = 2048
N_CORES = 8
P = 128
NCHUNK = 512  # token chunk (DoubleRow moving free dim = 2*NCHUNK fp8 = 512 bus elems)
S_H = 14.0    # fixed h quant scale (|h| stays < ~11; range to 240/14 = 17)


# ---------------------------------------------------------------- wait split
def _split_excess_waits(nc, limit=1):
    """This walrus build rejects >1 sync-wait command per instruction.
    Move excess waits onto fresh same-engine NOPs inserted just before."""
    template = bass.Bass(target_bir_lowering=False).sync.nop(nofuse=True).ins
    ctr = 0
    for bb in nc.main_func.blocks:
        out = []
        changed = False
        for ins in bb.instructions:
            si = ins.sync_info
            if si is not None and si.on_wait and len(si.on_wait) > limit:
                waits = list(si.on_wait)
                for w in waits[:-limit]:
                    ctr += 1
                    nop = copy.deepcopy(template)
                    nop.name = f"I-wsplit-{ctr}"
                    nop.engine = ins.engine
                    nop.bass_nofuse = True
                    nop.sync_info = mybir.SyncInfo(on_wait=[w], on_update=[])
                    nc.register_instruction(nop, overwrite=True)
                    out.append(nop)
                ins.sync_info = mybir.SyncInfo(
                    on_wait=waits[-limit:], on_update=list(si.on_update)
                )
                changed = True
            out.append(ins)
        if changed:
            bb.instructions = out
    return ctr


# ---------------------------------------------------------------- routing
def _gate_logits(x, gate_w):
    # Match the reference's jax-f32 CPU matmul as closely as possible.
    try:
        import jax
        import jax.numpy as jnp

        cpu = jax.devices("cpu")[0]
        with jax.default_device(cpu):
            return np.asarray(jnp.matmul(jnp.asarray(x), jnp.asarray(gate_w)))
    except Exception:
        return (x @ gate_w).astype(np.float32)


def _route(x, gate_w, e_bias):
    logits = _gate_logits(x, gate_w)  # [T, E] f32
    scores = (1.0 / (1.0 + np.exp(-logits))).astype(np.float32)
    sfc = scores + e_bias[None, :]
    grp = sfc.reshape(T, N_GROUP, E // N_GROUP)
    group_scores = np.sort(grp, axis=-1)[:, :, -2:].sum(-1)  # [T, G]
    group_idx = np.argsort(-group_scores, axis=-1, kind="stable")[:, :TOPK_GROUP]
    group_mask = np.zeros((T, N_GROUP), bool)
    group_mask[np.arange(T)[:, None], group_idx] = True
    expert_mask = np.repeat(group_mask, E // N_GROUP, axis=1)
    masked = np.where(expert_mask, sfc, -np.inf)
    topk_idx = np.argsort(-masked, axis=-1, kind="stable")[:, :TOP_K]  # [T, 4]
    topk_w = np.take_along_axis(scores, topk_idx, axis=1)
    topk_w = topk_w / topk_w.sum(axis=1, keepdims=True)
    return topk_idx.astype(np.int64), topk_w.astype(np.float32)


# ---------------------------------------------------------------- quantize
def _q8(a, scale):
    """e4m3-quantize a*scale (returns fp8 array); clip to TRN max 240."""
    return np.clip(a * np.float32(scale), -240.0, 240.0).astype(E4NP)


def _hilo(a, scale):
    hi = _q8(a, scale)
    lo = _q8(a - hi.astype(np.float32) / np.float32(scale), scale)
    return hi, lo


# ---------------------------------------------------------------- program
_PROGRAM_CACHE = {}


def _load_x(nc, xt_pool, xh_h, xl_h, C, spread=1, chunk_major=False):
    """Resident x^T hi/lo tiles [p, kblock, tok], loaded in 2-kblock pieces
    (hi first — the first 2/3 of each psum chain reads only hi).

    spread=1 emits all DMAs now; spread>1 returns (tiles, emit) where
    emit() issues the next batch of pieces — call it at successive points
    so a bulk X load doesn't push later weight DMAs back in the queue.
    chunk_major=True orders pieces by token chunk (all k-blocks of chunk 0
    first) so the first psum chains can start before the whole X lands."""
    n_d = D // P
    xh_t = xt_pool.tile([P, n_d, C], DT.float8e4, name="xh")
    xl_t = xt_pool.tile([P, n_d, C], DT.float8e4, name="xl")
    src_h = xh_h[:, :].rearrange("(k p) t -> p k t", p=P)
    src_l = xl_h[:, :].rearrange("(k p) t -> p k t", p=P)
    if chunk_major and C > NCHUNK:
        spans = [(o, min(NCHUNK, C - o)) for o in range(0, C, NCHUNK)]
        pieces = []
        for o, sz in spans:
            pieces += [(xh_t, src_h, d, o, sz) for d in range(0, n_d, 4)]
            pieces += [(xl_t, src_l, d, o, sz) for d in range(0, n_d, 4)]
    else:
        pieces = [(xh_t, src_h, d, 0, C) for d in range(0, n_d, 2)]
        pieces += [(xl_t, src_l, d, 0, C) for d in range(0, n_d, 2)]

    dk = 4 if (chunk_major and C > NCHUNK) else 2
    if spread <= 1:
        for dst, src, d, o, sz in pieces:
            nc.sync.dma_start(dst[:, d : d + dk, o : o + sz],
                              src[:, d : d + dk, o : o + sz])
        return xh_t, xl_t

    per = (len(pieces) + spread - 1) // spread
    it = iter(pieces)

    def emit():
        for _ in range(per):
            nxt = next(it, None)
            if nxt is None:
                return
            dst, src, d, o, sz = nxt
            nc.sync.dma_start(dst[:, d : d + dk, o : o + sz],
                              src[:, d : d + dk, o : o + sz])

    return (xh_t, xl_t), emit


class _Slot:
    """One expert slot: y = (silu(x@W1g) * x@W1u) @ W2 [* wr].

    Emits work in composable steps so slots can be interleaved:
      gate(u): one g/u output-tile pair of the gate_up matmul + h quant
      down(d2): one 128-row output tile of the down-proj + y writeback
    """

    def __init__(self, nc, pools, x_t, w1_h, w2_h, wr_h, y_h, C, twoI,
                 inv_g, c_u, c_y, w1_pre=None):
        self.nc = nc
        self.pools = pools
        self.xh_t, self.xl_t = x_t
        self.w1_h, self.w2_h, self.y_h = w1_h, w2_h, y_h
        self.C, self.twoI = C, twoI
        self.inv_g, self.c_u, self.c_y = inv_g, c_u, c_y
        self.w1_pre = w1_pre
        self.n_d = D // P
        self.n_i = twoI // P
        self.n_h = self.n_i // 2
        self.chunks = [(o, min(NCHUNK, C - o)) for o in range(0, C, NCHUNK)]
        (self.w1_pool, self.w2_pool, self.sg_pool, self.hf_pool, h_pool,
         self.y_pool, wr_pool, self.ps_g, self.ps_u, self.ps_dn) = pools
        self.wr_t = None
        if wr_h is not None:
            self.wr_t = wr_pool.tile([P, C], DT.float32, name="wr")
            nc.sync.dma_start(self.wr_t[:], wr_h[:, :])
        self.hh_t = h_pool.tile([P, self.n_h, C], DT.float8e4, name="hh")
        self.hl_t = h_pool.tile([P, self.n_h, C], DT.float8e4, name="hl")

    def gate(self, u):
        nc = self.nc
        n_d, n_h = self.n_d, self.n_h
        if u == 0 and self.w1_pre is not None:
            w1g, w1u = self.w1_pre
        else:
            w1g = self.w1_pool.tile([P, 2, n_d, P], DT.float8e4, name="w1g")
            nc.sync.dma_start(w1g[:], self.w1_h[u])
            w1u = self.w1_pool.tile([P, 2, n_d, P], DT.float8e4, name="w1u")
            nc.sync.dma_start(w1u[:], self.w1_h[n_h + u])
        for off, sz in self.chunks:
            psg = self.ps_g.tile([P, NCHUNK], DT.float32, name="psg")
            psu = self.ps_u.tile([P, NCHUNK], DT.float32, name="psu")
            for ps, wt in ((psg, w1g), (psu, w1u)):
                step = 0
                for wi, xt in ((0, self.xh_t), (1, self.xh_t), (0, self.xl_t)):
                    for kp in range(n_d // 2):
                        nc.tensor.matmul(
                            ps[:, :sz],
                            wt[:, wi, 2 * kp : 2 * kp + 2, :],
                            xt[:, 2 * kp : 2 * kp + 2, off : off + sz],
                            start=(step == 0),
                            stop=(step == 3 * (n_d // 2) - 1),
                            perf_mode=DR,
                        )
                        step += 1
            sgt = self.sg_pool.tile([P, NCHUNK], DT.float32, name="sgt")
            nc.scalar.activation(sgt[:, :sz], psg[:, :sz], AF.Sigmoid,
                                 scale=self.inv_g)
            gt = self.sg_pool.tile([P, NCHUNK], DT.float32, name="gt")
            nc.vector.scalar_tensor_tensor(
                gt[:, :sz], psg[:, :sz], self.inv_g, sgt[:, :sz],
                op0=MUL, op1=MUL
            )
            hf = self.hf_pool.tile([P, NCHUNK], DT.float32, name="hf")
            nc.vector.scalar_tensor_tensor(
                hf[:, :sz], psu[:, :sz], self.c_u, gt[:, :sz], op0=MUL, op1=MUL
            )
            nc.vector.tensor_copy(self.hh_t[:, u, off : off + sz], hf[:, :sz])
            nc.vector.tensor_sub(
                self.hl_t[:, u, off : off + sz], hf[:, :sz],
                self.hh_t[:, u, off : off + sz]
            )

    def down(self, d2):
        nc = self.nc
        n_hb = self.n_h
        w2t = self.w2_pool.tile([P, 2, n_hb, P], DT.float8e4, name="w2t")
        nc.sync.dma_start(w2t[:], self.w2_h[d2])
        yt = self.y_pool.tile([P, self.C], DT.bfloat16, name="yt")
        for off, sz in self.chunks:
            ps2 = self.ps_dn.tile([P, NCHUNK], DT.float32, name="psd")
            step = 0
            for wi, ht in ((0, self.hh_t), (1, self.hh_t), (0, self.hl_t)):
                for hp in range(n_hb // 2):
                    nc.tensor.matmul(
                        ps2[:, :sz],
                        w2t[:, wi, 2 * hp : 2 * hp + 2, :],
                        ht[:, 2 * hp : 2 * hp + 2, off : off + sz],
                        start=(step == 0),
                        stop=(step == 3 * (n_hb // 2) - 1),
                        perf_mode=DR,
                    )
                    step += 1
            if self.wr_t is not None:
                nc.vector.tensor_mul(
                    yt[:, off : off + sz], ps2[:, :sz],
                    self.wr_t[:, off : off + sz]
                )
            else:
                nc.scalar.activation(
                    yt[:, off : off + sz], ps2[:, :sz], AF.Copy, scale=self.c_y
                )
        nc.sync.dma_start(self.y_h[d2 * P : (d2 + 1) * P, :], yt[:])


def _build_program(C1, C2, C3, inv_g, c_u, c_y):
    key = (C1, C2, C3, round(float(inv_g), 10), round(float(c_u), 10),
           round(float(c_y), 10))
    if key in _PROGRAM_CACHE:
        return _PROGRAM_CACHE[key]

    nc = bass.Bass(target_bir_lowering=False)
    TS = T // N_CORES  # shared tokens per core

    def din(name, shape, dt=DT.float8e4):
        return nc.dram_tensor(name, shape, dt, kind="ExternalInput")

    xt1h = din("xt1h", [D, C1]); xt1l = din("xt1l", [D, C1])
    xt2h = din("xt2h", [D, C2]); xt2l = din("xt2l", [D, C2])
    xtch = din("xtch", [D, C3]); xtcl = din("xtcl", [D, C3])
    xtsh = din("xtsh", [D, TS]); xtsl = din("xtsl", [D, TS])
    w1a = din("w1a", [2 * I // P, P, 2, D // P, P])
    w2a = din("w2a", [D // P, P, 2, I // P, P])
    w1b = din("w1b", [2 * I // P, P, 2, D // P, P])
    w2b = din("w2b", [D // P, P, 2, I // P, P])
    w1c = din("w1c", [2 * I // P, P, 2, D // P, P])
    w2c = din("w2c", [D // P, P, 2, I // P, P])
    ws1 = din("ws1", [2 * SHARED_I // P, P, 2, D // P, P])
    ws2 = din("ws2", [D // P, P, 2, SHARED_I // P, P])
    wr1 = din("wr1", [P, C1], DT.float32)
    wr2 = din("wr2", [P, C2], DT.float32)
    wrc = din("wrc", [P, C3], DT.float32)
    y1 = nc.dram_tensor("y1", [D, C1], DT.bfloat16, kind="ExternalOutput")
    y2 = nc.dram_tensor("y2", [D, C2], DT.bfloat16, kind="ExternalOutput")
    yc = nc.dram_tensor("yc", [D, C3], DT.bfloat16, kind="ExternalOutput")
    ys = nc.dram_tensor("ys", [D, TS], DT.bfloat16, kind="ExternalOutput")

    with tile.TileContext(nc) as tc:
        with (
            tc.tile_pool(name="xt", bufs=2) as xt_pool,
            tc.tile_pool(name="xts", bufs=1) as xts_pool,
            tc.tile_pool(name="xtc", bufs=1) as xtc_pool,
            tc.tile_pool(name="w1p", bufs=3) as w1_pool,
            tc.tile_pool(name="w2p", bufs=5) as w2_pool,
            tc.tile_pool(name="sgp", bufs=2) as sg_pool,
            tc.tile_pool(name="hfp", bufs=2) as hf_pool,
            tc.tile_pool(name="hp", bufs=2) as h_pool,
            tc.tile_pool(name="yp", bufs=4) as y_pool,
            tc.tile_pool(name="wrp", bufs=3) as wr_pool,
            tc.tile_pool(name="psg", bufs=2, space="PSUM") as ps_g,
            tc.tile_pool(name="psu", bufs=3, space="PSUM") as ps_u,
            tc.tile_pool(name="psdn", bufs=3, space="PSUM") as ps_dn,
        ):
            pools = (w1_pool, w2_pool, sg_pool, hf_pool, h_pool,
                     y_pool, wr_pool, ps_g, ps_u, ps_dn)
            n_h1 = I // P
            n_hs = SHARED_I // P
            # startup: first weight pair of slot A ahead of the bulk X load
            # (few large DMAs: HWDGE issue overhead dominates at startup)
            w1g0 = w1_pool.tile([P, 2, D // P, P], DT.float8e4, name="w1g")
            nc.sync.dma_start(w1g0[:], w1a[0])
            w1u0 = w1_pool.tile([P, 2, D // P, P], DT.float8e4, name="w1u")
            nc.sync.dma_start(w1u0[:], w1a[I // P])
            xA = _load_x(nc, xt_pool, xt1h, xt1l, C1)
            sA = _Slot(nc, pools, xA, w1a, w2a, wr1, y1, C1, 2 * I,
                       inv_g, c_u, c_y, w1_pre=(w1g0, w1u0))
            xB = emitB = None
            for u in range(n_h1):
                if u == 2:
                    xB, emitB = _load_x(nc, xt_pool, xt2h, xt2l, C2, spread=5)
                if emitB is not None and 2 <= u < 7:
                    emitB()
                sA.gate(u)
            sB = _Slot(nc, pools, xB, w1b, w2b, wr2, y2, C2, 2 * I,
                       inv_g, c_u, c_y)
            for d2 in range(D // P):
                if d2 == 2:
                    xC = _load_x(nc, xtc_pool, xtch, xtcl, C3)
                sA.down(d2)
            sC = _Slot(nc, pools, xC, w1c, w2c, wrc, yc, C3, 2 * I,
                       inv_g, c_u, c_y)
            # overflow slot C rides along with slot B (B's phases have DMA
            # slack; the shared phase is already at DMA capacity)
            xS = emitS = None
            for u in range(n_h1):
                if u == 1:
                    xS, emitS = _load_x(nc, xts_pool, xtsh, xtsl, TS, spread=5)
                if emitS is not None and 1 <= u < 6:
                    emitS()
                sB.gate(u)
                sC.gate(u)
            sS = _Slot(nc, pools, xS, ws1, ws2, None, ys, TS, 2 * SHARED_I,
                       inv_g, c_u, c_y)
            for d2 in range(D // P):
                sB.down(d2)
                sC.down(d2)
            for u in range(n_hs):
                sS.gate(u)
            for d2 in range(D // P):
                sS.down(d2)

    _split_excess_waits(nc, limit=1)
    _PROGRAM_CACHE[key] = nc
    return nc


def _solve_caps(counts):
    """Pick slot capacities (C1, C2, C3) minimizing C1+C2+C3 where the 8
    largest experts live in slot A (cap C1), the 8 smallest in slot B
    (cap C2), and overflow is cut into <= N_CORES pieces of <= C3 tokens."""
    s = np.sort(counts)[::-1]
    A, B = s[:N_CORES], s[N_CORES:]
    best = (int(A.max() + B.max()) + 16, int(A.max()), int(B.max()), 16)
    for C1 in range(1024, int(A.max()) + 1, 2):
        ovA = np.maximum(A - C1, 0)
        for C2 in range(768, int(B.max()) + 1, 2):
            ov = np.concatenate([ovA, np.maximum(B - C2, 0)])
            ov = ov[ov > 0]
            if len(ov) == 0:
                cand = (C1 + C2 + 16, C1, C2, 16)
            elif len(ov) > N_CORES:
                continue  # even one piece per overflowing expert won't fit
            else:
                lo = max(16, int(np.ceil(ov.sum() / N_CORES)))
                hi = max(lo, int(ov.max()))
                while lo < hi:
                    mid = (lo + hi) // 2
                    if np.ceil(ov / mid).sum() <= N_CORES:
                        hi = mid
                    else:
                        lo = mid + 1
                cand = (C1 + C2 + lo, C1, C2, lo)
            if cand[0] < best[0]:
                best = cand
    return best[1], best[2], best[3]


# ---------------------------------------------------------------- packing
def _pack_w1(w, twoI, s_w):
    """w [D, twoI] f32 -> [twoI/P, P, 2, D/P, P] fp8 (dim2 = hi/lo)."""
    n_d, n_i = D // P, twoI // P
    hi, lo = _hilo(w.astype(np.float32), s_w)

    def pk(a):
        return a.reshape(n_d, P, n_i, P).transpose(2, 1, 0, 3)

    return np.ascontiguousarray(np.stack([pk(hi), pk(lo)], axis=2))


def _pack_w2(w, I_, s_w):
    """w [I_, D] f32 -> [D/P, P, 2, I_/P, P] fp8 (dim2 = hi/lo)."""
    n_h, n_d2 = I_ // P, D // P
    hi, lo = _hilo(w.astype(np.float32), s_w)

    def pk(a):
        return a.reshape(n_h, P, n_d2, P).transpose(2, 1, 0, 3)

    return np.ascontiguousarray(np.stack([pk(hi), pk(lo)], axis=2))


# ---------------------------------------------------------------- kernel
def _prepare(hidden_states, gate_w, e_bias, w_gate_up, w_down, ws_gate_up, ws_down):
    x = np.asarray(hidden_states, dtype=np.float32)
    topk_idx, topk_w = _route(x, np.asarray(gate_w), np.asarray(e_bias))

    # dispatch: token lists per expert, sorted-stable by expert id
    flat_e = topk_idx.ravel()
    order = np.argsort(flat_e, kind="stable")
    pair_tok = order // TOP_K
    counts = np.bincount(flat_e, minlength=E)
    starts = np.zeros(E + 1, np.int64)
    np.cumsum(counts, out=starts[1:])

    # expert -> core assignment: pair largest with smallest; overflow beyond
    # the slot caps is cut into per-core pieces handled by slot C
    by_count = np.argsort(-counts, kind="stable")
    slotA = by_count[:N_CORES]
    slotB = by_count[E - 1 : N_CORES - 1 : -1]  # reversed smallest half
    C1, C2, C3 = _solve_caps(counts)

    # overflow pieces: (expert, local_start, n_tokens), each <= C3
    pieces = []
    cap_of = {}
    for c in range(N_CORES):
        cap_of[int(slotA[c])] = C1
        cap_of[int(slotB[c])] = C2
    for e_id in range(E):
        ov = int(counts[e_id]) - cap_of[e_id]
        o = cap_of[e_id]
        while ov > 0:
            take = min(ov, C3)
            pieces.append((e_id, o, take))
            o += take
            ov -= take
    assert len(pieces) <= N_CORES, (pieces, C1, C2, C3)

    # global quantization scales -> SPMD-safe immediate dequant factors
    w_gate_up = np.asarray(w_gate_up, dtype=np.float32)
    w_down = np.asarray(w_down, dtype=np.float32)
    ws_gate_up = np.asarray(ws_gate_up, dtype=np.float32)
    ws_down = np.asarray(ws_down, dtype=np.float32)
    s_x = 224.0 / max(float(np.abs(x).max()), 1e-30)
    s_w1 = 224.0 / max(float(np.abs(w_gate_up).max()),
                       float(np.abs(ws_gate_up).max()), 1e-30)
    s_w2 = 224.0 / max(float(np.abs(w_down).max()),
                       float(np.abs(ws_down).max()), 1e-30)
    inv_g = 1.0 / (s_x * s_w1)          # dequant of gate_up psum
    c_u = S_H / (s_x * s_w1)            # dequant of u * h-quant scale
    c_y = 1.0 / (S_H * s_w2)            # dequant of down psum (shared)
    pair_w = (topk_w.ravel()[order] * np.float32(ROUTED_SCALE * c_y)).astype(
        np.float32
    )

    nc = _build_program(C1, C2, C3, inv_g, c_u, c_y)

    xT = np.ascontiguousarray(x.T)  # [D, T]
    xTh, xTl = _hilo(xT, s_x)

    w1_p = {int(e): _pack_w1(w_gate_up[e], 2 * I, s_w1) for e in range(E)}
    w2_p = {int(e): _pack_w2(w_down[e], I, s_w2) for e in range(E)}
    ws1_p = _pack_w1(ws_gate_up, 2 * SHARED_I, s_w1)
    ws2_p = _pack_w2(ws_down, SHARED_I, s_w2)

    def slot_inputs(m, e_id, lo, n_e, C, xh_name, xl_name, wr_name):
        idx = pair_tok[starts[e_id] + lo : starts[e_id] + lo + n_e]
        w = pair_w[starts[e_id] + lo : starts[e_id] + lo + n_e]
        idx_pad = np.zeros(C, np.int64)
        idx_pad[:n_e] = idx
        w_pad = np.zeros(C, np.float32)
        w_pad[:n_e] = w
        m[xh_name] = xTh[:, idx_pad]
        m[xl_name] = xTl[:, idx_pad]
        m[wr_name] = np.ascontiguousarray(np.broadcast_to(w_pad, (P, C)))
        return (idx, n_e)

    TS = T // N_CORES
    in_maps = []
    core_info = []
    for c in range(N_CORES):
        eA, eB = int(slotA[c]), int(slotB[c])
        m = {}
        info = []
        info.append(slot_inputs(m, eA, 0, min(int(counts[eA]), C1), C1,
                                "xt1h", "xt1l", "wr1"))
        info.append(slot_inputs(m, eB, 0, min(int(counts[eB]), C2), C2,
                                "xt2h", "xt2l", "wr2"))
        if c < len(pieces):
            eC, lo, n_c = pieces[c]
        else:
            eC, lo, n_c = eA, 0, 0   # idle slot C: wr=0 zeroes the output
        info.append(slot_inputs(m, eC, lo, n_c, C3, "xtch", "xtcl", "wrc"))
        m["xtsh"] = np.ascontiguousarray(xTh[:, c * TS : (c + 1) * TS])
        m["xtsl"] = np.ascontiguousarray(xTl[:, c * TS : (c + 1) * TS])
        m["w1a"] = w1_p[eA]
        m["w2a"] = w2_p[eA]
        m["w1b"] = w1_p[eB]
        m["w2b"] = w2_p[eB]
        m["w1c"] = w1_p[eC]
        m["w2c"] = w2_p[eC]
        m["ws1"] = ws1_p
        m["ws2"] = ws2_p
        in_maps.append(m)
        core_info.append(info)
    return nc, in_maps, core_info


def _combine(res_results, core_info):
    TS = T // N_CORES
    out = np.zeros((T, D), np.float32)
    for c in range(N_CORES):
        (idxA, nA), (idxB, nB), (idxC, nC) = core_info[c]
        out[idxA] += res_results[c]["y1"][:, :nA].T
        out[idxB] += res_results[c]["y2"][:, :nB].T
        if nC:
            out[idxC] += res_results[c]["yc"][:, :nC].T
        out[c * TS : (c + 1) * TS] += res_results[c]["ys"].T
    return out


def kernel(hidden_states, gate_w, e_bias, w_gate_up, w_down, ws_gate_up, ws_down):
    nc, in_maps, core_info = _prepare(
        hidden_states, gate_w, e_bias, w_gate_up, w_down, ws_gate_up, ws_down
    )
    res = run_bass_kernel_spmd(nc, in_maps, list(range(N_CORES)))
    return _combine(res.results, core_info)


# revision 34
# speedup vs baseline: 1.4151x; 1.0014x over previous
"""DeepseekV2 MoE layer on 8 Trainium2 NeuronCores.

Strategy (expert-parallel, per the sharding hint):
  - Router gate + grouped top-k computed on host (0.03% of module FLOPs);
    it determines the dispatch, which IS the input sharding.
  - 16 routed experts paired big-count-with-small-count onto 8 cores
    (2 experts per core, token lists gathered host-side, padded to a
    shared per-slot capacity so all cores run one SPMD program).
    Token overflow beyond the slot caps is cut into <= 8 pieces handled
    by a third mini-slot per core with per-core expert weights (solved
    host-side to minimize total padded capacity).
  - Shared-expert MLP is data-parallel over tokens: each core runs
    T/8 = 512 tokens through the full shared MLP.
  - All matmuls run as fp8-e4m3 DoubleRow (2 K-blocks per instruction at
    0.5 cycles/row = 4x bf16 PE throughput), with 3-term error
    compensation per matmul:  X@W ~= Xh@Wh + Xh@Wl + Xl@Wh  where
    Xh/Wh are e4m3 quantizations and Xl/Wl the e4m3-quantized residuals.
    Net PE cost is 0.75x bf16 for ~2e-3 rel error (vs 4e-3 for bf16).
  - Quantization scales are global (shared across experts/cores) so all
    dequant factors are compile-time immediates in the single SPMD
    program; per-token routing weights fold the down-proj dequant.
  - x and all weights are quantized host-side; only the MLP intermediate
    h is quantized (hi+lo) on device.
"""

import sys

sys.path.insert(0, "/opt/trn_rl_repo")

import copy

import ml_dtypes
import numpy as np

import concourse.bass as bass
import concourse.mybir as mybir
import concourse.tile as tile
from concourse.bass_utils import run_bass_kernel_spmd

DT = mybir.dt
E4NP = ml_dtypes.float8_e4m3  # TRN float8e4: max-normal 240
AF = mybir.ActivationFunctionType
DR = mybir.MatmulPerfMode.DoubleRow
MUL = mybir.AluOpType.mult

T, D, E, I = 4096, 2048, 16, 1024
TOP_K, N_GROUP, TOPK_GROUP = 4, 4, 2
ROUTED_SCALE = 2.5
SHARED_I = 2048
N_CORES = 8
P = 128
NCHUNK = 512  # token chunk (DoubleRow moving free dim = 2*NCHUNK fp8 = 512 bus elems)
S_H = 14.0    # fixed h quant scale (|h| stays < ~11; range to 240/14 = 17)


# ---------------------------------------------------------------- wait split
def _split_excess_waits(nc, limit=1):
    """This walrus build rejects >1 sync-wait command per instruction.
    Move excess waits onto fresh same-engine NOPs inserted just before."""
    template = bass.Bass(target_bir_lowering=False).sync.nop(nofuse=True).ins
    ctr = 0
    for bb in nc.main_func.blocks:
        out = []
        changed = False
        for ins in bb.instructions:
            si = ins.sync_info
            if si is not None and si.on_wait and len(si.on_wait) > limit:
                waits = list(si.on_wait)
                for w in waits[:-limit]:
                    ctr += 1
                    nop = copy.deepcopy(template)
                    nop.name = f"I-wsplit-{ctr}"
                    nop.engine = ins.engine
                    nop.bass_nofuse = True
                    nop.sync_info = mybir.SyncInfo(on_wait=[w], on_update=[])
                    nc.register_instruction(nop, overwrite=True)
                    out.append(nop)
                ins.sync_info = mybir.SyncInfo(
                    on_wait=waits[-limit:], on_update=list(si.on_update)
                )
                changed = True
            out.append(ins)
        if changed:
            bb.instructions = out
    return ctr


# ---------------------------------------------------------------- routing
def _gate_logits(x, gate_w):
    # Match the reference's jax-f32 CPU matmul as closely as possible.
    try:
        import jax
        import jax.numpy as jnp

        cpu = jax.devices("cpu")[0]
        with jax.default_device(cpu):
            return np.asarray(jnp.matmul(jnp.asarray(x), jnp.asarray(gate_w)))
    except Exception:
        return (x @ gate_w).astype(np.float32)


def _route(x, gate_w, e_bias):
    logits = _gate_logits(x, gate_w)  # [T, E] f32
    scores = (1.0 / (1.0 + np.exp(-logits))).astype(np.float32)
    sfc = scores + e_bias[None, :]
    grp = sfc.reshape(T, N_GROUP, E // N_GROUP)
    group_scores = np.sort(grp, axis=-1)[:, :, -2:].sum(-1)  # [T, G]
    group_idx = np.argsort(-group_scores, axis=-1, kind="stable")[:, :TOPK_GROUP]
    group_mask = np.zeros((T, N_GROUP), bool)
    group_mask[np.arange(T)[:, None], group_idx] = True
    expert_mask = np.repeat(group_mask, E // N_GROUP, axis=1)
    masked = np.where(expert_mask, sfc, -np.inf)
    topk_idx = np.argsort(-masked, axis=-1, kind="stable")[:, :TOP_K]  # [T, 4]
    topk_w = np.take_along_axis(scores, topk_idx, axis=1)
    topk_w = topk_w / topk_w.sum(axis=1, keepdims=True)
    return topk_idx.astype(np.int64), topk_w.astype(np.float32)


# ---------------------------------------------------------------- quantize
def _q8(a, scale):
    """e4m3-quantize a*scale (returns fp8 array); clip to TRN max 240."""
    return np.clip(a * np.float32(scale), -240.0, 240.0).astype(E4NP)


def _hilo(a, scale):
    hi = _q8(a, scale)
    lo = _q8(a - hi.astype(np.float32) / np.float32(scale), scale)
    return hi, lo


# ---------------------------------------------------------------- program
_PROGRAM_CACHE = {}


def _load_x(nc, xt_pool, xh_h, xl_h, C, spread=1, chunk_major=False):
    """Resident x^T hi/lo tiles [p, kblock, tok], loaded in 2-kblock pieces
    (hi first — the first 2/3 of each psum chain reads only hi).

    spread=1 emits all DMAs now; spread>1 returns (tiles, emit) where
    emit() issues the next batch of pieces — call it at successive points
    so a bulk X load doesn't push later weight DMAs back in the queue.
    chunk_major=True orders pieces by token chunk (all k-blocks of chunk 0
    first) so the first psum chains can start before the whole X lands."""
    n_d = D // P
    xh_t = xt_pool.tile([P, n_d, C], DT.float8e4, name="xh")
    xl_t = xt_pool.tile([P, n_d, C], DT.float8e4, name="xl")
    src_h = xh_h[:, :].rearrange("(k p) t -> p k t", p=P)
    src_l = xl_h[:, :].rearrange("(k p) t -> p k t", p=P)
    if chunk_major and C > NCHUNK:
        spans = [(o, min(NCHUNK, C - o)) for o in range(0, C, NCHUNK)]
        pieces = []
        for o, sz in spans:
            pieces += [(xh_t, src_h, d, o, sz) for d in range(0, n_d, 4)]
            pieces += [(xl_t, src_l, d, o, sz) for d in range(0, n_d, 4)]
    else:
        pieces = [(xh_t, src_h, d, 0, C) for d in range(0, n_d, 2)]
        pieces += [(xl_t, src_l, d, 0, C) for d in range(0, n_d, 2)]

    dk = 4 if (chunk_major and C > NCHUNK) else 2
    if spread <= 1:
        for dst, src, d, o, sz in pieces:
            nc.sync.dma_start(dst[:, d : d + dk, o : o + sz],
                              src[:, d : d + dk, o : o + sz])
        return xh_t, xl_t

    per = (len(pieces) + spread - 1) // spread
    it = iter(pieces)

    def emit():
        for _ in range(per):
            nxt = next(it, None)
            if nxt is None:
                return
            dst, src, d, o, sz = nxt
            nc.sync.dma_start(dst[:, d : d + dk, o : o + sz],
                              src[:, d : d + dk, o : o + sz])

    return (xh_t, xl_t), emit


class _Slot:
    """One expert slot: y = (silu(x@W1g) * x@W1u) @ W2 [* wr].

    Emits work in composable steps so slots can be interleaved:
      gate(u): one g/u output-tile pair of the gate_up matmul + h quant
      down(d2): one 128-row output tile of the down-proj + y writeback
    """

    def __init__(self, nc, pools, x_t, w1_h, w2_h, wr_h, y_h, C, twoI,
                 inv_g, c_u, c_y, w1_pre=None, ps_override=None):
        self.nc = nc
        self.pools = pools
        self.xh_t, self.xl_t = x_t
        self.w1_h, self.w2_h, self.y_h = w1_h, w2_h, y_h
        self.C, self.twoI = C, twoI
        self.inv_g, self.c_u, self.c_y = inv_g, c_u, c_y
        self.w1_pre = w1_pre
        self.n_d = D // P
        self.n_i = twoI // P
        self.n_h = self.n_i // 2
        self.chunks = [(o, min(NCHUNK, C - o)) for o in range(0, C, NCHUNK)]
        (self.w1_pool, self.w2_pool, self.sg_pool, self.hf_pool, h_pool,
         self.y_pool, wr_pool, self.ps_g, self.ps_u, self.ps_dn) = pools
        if ps_override is not None:
            self.ps_g, self.ps_u, self.ps_dn = ps_override
        self.wr_t = None
        if wr_h is not None:
            self.wr_t = wr_pool.tile([P, C], DT.float32, name="wr")
            nc.sync.dma_start(self.wr_t[:], wr_h[:, :])
        self.hh_t = h_pool.tile([P, self.n_h, C], DT.float8e4, name="hh")
        self.hl_t = h_pool.tile([P, self.n_h, C], DT.float8e4, name="hl")

    def gate(self, u):
        nc = self.nc
        n_d, n_h = self.n_d, self.n_h
        if u == 0 and self.w1_pre is not None:
            w1g, w1u = self.w1_pre
        else:
            w1g = self.w1_pool.tile([P, 2, n_d, P], DT.float8e4, name="w1g")
            nc.sync.dma_start(w1g[:], self.w1_h[u])
            w1u = self.w1_pool.tile([P, 2, n_d, P], DT.float8e4, name="w1u")
            nc.sync.dma_start(w1u[:], self.w1_h[n_h + u])
        for off, sz in self.chunks:
            psg = self.ps_g.tile([P, NCHUNK], DT.float32, name="psg")
            psu = self.ps_u.tile([P, NCHUNK], DT.float32, name="psu")
            for ps, wt in ((psg, w1g), (psu, w1u)):
                step = 0
                for wi, xt in ((0, self.xh_t), (1, self.xh_t), (0, self.xl_t)):
                    for kp in range(n_d // 2):
                        nc.tensor.matmul(
                            ps[:, :sz],
                            wt[:, wi, 2 * kp : 2 * kp + 2, :],
                            xt[:, 2 * kp : 2 * kp + 2, off : off + sz],
                            start=(step == 0),
                            stop=(step == 3 * (n_d // 2) - 1),
                            perf_mode=DR,
                        )
                        step += 1
            sgt = self.sg_pool.tile([P, NCHUNK], DT.float32, name="sgt")
            nc.scalar.activation(sgt[:, :sz], psg[:, :sz], AF.Sigmoid,
                                 scale=self.inv_g)
            gt = self.sg_pool.tile([P, NCHUNK], DT.float32, name="gt")
            nc.vector.scalar_tensor_tensor(
                gt[:, :sz], psg[:, :sz], self.inv_g, sgt[:, :sz],
                op0=MUL, op1=MUL
            )
            hf = self.hf_pool.tile([P, NCHUNK], DT.float32, name="hf")
            nc.vector.scalar_tensor_tensor(
                hf[:, :sz], psu[:, :sz], self.c_u, gt[:, :sz], op0=MUL, op1=MUL
            )
            nc.vector.tensor_copy(self.hh_t[:, u, off : off + sz], hf[:, :sz])
            nc.vector.tensor_sub(
                self.hl_t[:, u, off : off + sz], hf[:, :sz],
                self.hh_t[:, u, off : off + sz]
            )

    def down(self, d2, fine_tail=False):
        nc = self.nc
        n_hb = self.n_h
        w2t = self.w2_pool.tile([P, 2, n_hb, P], DT.float8e4, name="w2t")
        nc.sync.dma_start(w2t[:], self.w2_h[d2])
        yt = self.y_pool.tile([P, self.C], DT.bfloat16, name="yt")
        if fine_tail:
            # halve the chunks and write y back per chunk so the final DMA
            # covers only the last half (shorter end-of-program tail)
            half = NCHUNK // 2
            chunks = [(o, min(half, self.C - o)) for o in range(0, self.C, half)]
        else:
            chunks = self.chunks
        for off, sz in chunks:
            ps2 = self.ps_dn.tile([P, NCHUNK], DT.float32, name="psd")
            step = 0
            for wi, ht in ((0, self.hh_t), (1, self.hh_t), (0, self.hl_t)):
                for hp in range(n_hb // 2):
                    nc.tensor.matmul(
                        ps2[:, :sz],
                        w2t[:, wi, 2 * hp : 2 * hp + 2, :],
                        ht[:, 2 * hp : 2 * hp + 2, off : off + sz],
                        start=(step == 0),
                        stop=(step == 3 * (n_hb // 2) - 1),
                        perf_mode=DR,
                    )
                    step += 1
            if self.wr_t is not None:
                nc.vector.tensor_mul(
                    yt[:, off : off + sz], ps2[:, :sz],
                    self.wr_t[:, off : off + sz]
                )
            else:
                nc.scalar.activation(
                    yt[:, off : off + sz], ps2[:, :sz], AF.Copy, scale=self.c_y
                )
            if fine_tail:
                nc.sync.dma_start(
                    self.y_h[d2 * P : (d2 + 1) * P, off : off + sz],
                    yt[:, off : off + sz],
                )
        if not fine_tail:
            nc.sync.dma_start(self.y_h[d2 * P : (d2 + 1) * P, :], yt[:])


def _build_program(C1, C2, C3, inv_g, c_u, c_y):
    key = (C1, C2, C3, round(float(inv_g), 10), round(float(c_u), 10),
           round(float(c_y), 10))
    if key in _PROGRAM_CACHE:
        return _PROGRAM_CACHE[key]

    nc = bass.Bass(target_bir_lowering=False)
    TS = T // N_CORES  # shared tokens per core

    def din(name, shape, dt=DT.float8e4):
        return nc.dram_tensor(name, shape, dt, kind="ExternalInput")

    xt1h = din("xt1h", [D, C1]); xt1l = din("xt1l", [D, C1])
    xt2h = din("xt2h", [D, C2]); xt2l = din("xt2l", [D, C2])
    xtch = din("xtch", [D, C3]); xtcl = din("xtcl", [D, C3])
    xtsh = din("xtsh", [D, TS]); xtsl = din("xtsl", [D, TS])
    w1a = din("w1a", [2 * I // P, P, 2, D // P, P])
    w2a = din("w2a", [D // P, P, 2, I // P, P])
    w1b = din("w1b", [2 * I // P, P, 2, D // P, P])
    w2b = din("w2b", [D // P, P, 2, I // P, P])
    w1c = din("w1c", [2 * I // P, P, 2, D // P, P])
    w2c = din("w2c", [D // P, P, 2, I // P, P])
    ws1 = din("ws1", [2 * SHARED_I // P, P, 2, D // P, P])
    ws2 = din("ws2", [D // P, P, 2, SHARED_I // P, P])
    wr1 = din("wr1", [P, C1], DT.float32)
    wr2 = din("wr2", [P, C2], DT.float32)
    wrc = din("wrc", [P, C3], DT.float32)
    y1 = nc.dram_tensor("y1", [D, C1], DT.bfloat16, kind="ExternalOutput")
    y2 = nc.dram_tensor("y2", [D, C2], DT.bfloat16, kind="ExternalOutput")
    yc = nc.dram_tensor("yc", [D, C3], DT.bfloat16, kind="ExternalOutput")
    ys = nc.dram_tensor("ys", [D, TS], DT.bfloat16, kind="ExternalOutput")

    with tile.TileContext(nc) as tc:
        with (
            tc.tile_pool(name="xt", bufs=2) as xt_pool,
            tc.tile_pool(name="xts", bufs=1) as xts_pool,
            tc.tile_pool(name="xtc", bufs=1) as xtc_pool,
            tc.tile_pool(name="w1p", bufs=3) as w1_pool,
            tc.tile_pool(name="w2p", bufs=5) as w2_pool,
            tc.tile_pool(name="sgp", bufs=2) as sg_pool,
            tc.tile_pool(name="hfp", bufs=2) as hf_pool,
            tc.tile_pool(name="hp", bufs=2) as h_pool,
            tc.tile_pool(name="yp", bufs=4) as y_pool,
            tc.tile_pool(name="wrp", bufs=3) as wr_pool,
            tc.tile_pool(name="psg", bufs=2, space="PSUM") as ps_g,
            tc.tile_pool(name="psu", bufs=3, space="PSUM") as ps_u,
            tc.tile_pool(name="psdn", bufs=3, space="PSUM") as ps_dn,
        ):
            pools = (w1_pool, w2_pool, sg_pool, hf_pool, h_pool,
                     y_pool, wr_pool, ps_g, ps_u, ps_dn)
            n_h1 = I // P
            n_hs = SHARED_I // P
            # startup: first weight pair of slot A ahead of the bulk X load
            # (few large DMAs: HWDGE issue overhead dominates at startup)
            w1g0 = w1_pool.tile([P, 2, D // P, P], DT.float8e4, name="w1g")
            nc.sync.dma_start(w1g0[:, :, : D // P // 2, :],
                              w1a[0, :, :, : D // P // 2, :])
            nc.sync.dma_start(w1g0[:, :, D // P // 2 :, :],
                              w1a[0, :, :, D // P // 2 :, :])
            xA, emitA0 = _load_x(nc, xt_pool, xt1h, xt1l, C1, spread=4)
            emitA0()  # first quarter of X (hi k-blocks) ahead of w1u0
            w1u0 = w1_pool.tile([P, 2, D // P, P], DT.float8e4, name="w1u")
            nc.sync.dma_start(w1u0[:], w1a[I // P])
            emitA0(); emitA0(); emitA0()
            sA = _Slot(nc, pools, xA, w1a, w2a, wr1, y1, C1, 2 * I,
                       inv_g, c_u, c_y, w1_pre=(w1g0, w1u0))
            xB = emitB = None
            for u in range(n_h1):
                if u == 2:
                    xB, emitB = _load_x(nc, xt_pool, xt2h, xt2l, C2, spread=5)
                if emitB is not None and 2 <= u < 7:
                    emitB()
                sA.gate(u)
            sB = _Slot(nc, pools, xB, w1b, w2b, wr2, y2, C2, 2 * I,
                       inv_g, c_u, c_y)
            for d2 in range(D // P):
                if d2 == 2:
                    xC = _load_x(nc, xtc_pool, xtch, xtcl, C3)
                sA.down(d2)
            sC = _Slot(nc, pools, xC, w1c, w2c, wrc, yc, C3, 2 * I,
                       inv_g, c_u, c_y)
            # overflow slot C rides along with slot B (B's phases have DMA
            # slack; the shared phase is already at DMA capacity)
            xS = emitS = None
            for u in range(n_h1):
                if u == 1:
                    xS, emitS = _load_x(nc, xts_pool, xtsh, xtsl, TS, spread=5)
                if emitS is not None and 1 <= u < 6:
                    emitS()
                sB.gate(u)
                sC.gate(u)
            sS = _Slot(nc, pools, xS, ws1, ws2, None, ys, TS, 2 * SHARED_I,
                       inv_g, c_u, c_y)
            for d2 in range(D // P):
                sB.down(d2)
                sC.down(d2)
            for u in range(n_hs):
                sS.gate(u)
            for d2 in range(D // P):
                sS.down(d2, fine_tail=(d2 == D // P - 1))

    _split_excess_waits(nc, limit=1)
    _PROGRAM_CACHE[key] = nc
    return nc


def _solve_caps(counts):
    """Pick slot capacities (C1, C2, C3) minimizing C1+C2+C3 where the 8
    largest experts live in slot A (cap C1), the 8 smallest in slot B
    (cap C2), and overflow is cut into <= N_CORES pieces of <= C3 tokens."""
    s = np.sort(counts)[::-1]
    A, B = s[:N_CORES], s[N_CORES:]
    best = (int(A.max() + B.max()) + 16, int(A.max()), int(B.max()), 16)
    for C1 in range(1024, int(A.max()) + 1, 2):
        ovA = np.maximum(A - C1, 0)
        for C2 in range(768, int(B.max()) + 1, 2):
            ov = np.concatenate([ovA, np.maximum(B - C2, 0)])
            ov = ov[ov > 0]
            if len(ov) == 0:
                cand = (C1 + C2 + 16, C1, C2, 16)
            elif len(ov) > N_CORES:
                continue  # even one piece per overflowing expert won't fit
            else:
                lo = max(16, int(np.ceil(ov.sum() / N_CORES)))
                hi = max(lo, int(ov.max()))
                while lo < hi:
                    mid = (lo + hi) // 2
                    if np.ceil(ov / mid).sum() <= N_CORES:
                        hi = mid
                    else:
                        lo = mid + 1
                cand = (C1 + C2 + lo, C1, C2, lo)
            if cand[0] < best[0]:
                best = cand
    return best[1], best[2], best[3]


# ---------------------------------------------------------------- packing
def _pack_w1(w, twoI, s_w):
    """w [D, twoI] f32 -> [twoI/P, P, 2, D/P, P] fp8 (dim2 = hi/lo)."""
    n_d, n_i = D // P, twoI // P
    hi, lo = _hilo(w.astype(np.float32), s_w)

    def pk(a):
        return a.reshape(n_d, P, n_i, P).transpose(2, 1, 0, 3)

    return np.ascontiguousarray(np.stack([pk(hi), pk(lo)], axis=2))


def _pack_w2(w, I_, s_w):
    """w [I_, D] f32 -> [D/P, P, 2, I_/P, P] fp8 (dim2 = hi/lo)."""
    n_h, n_d2 = I_ // P, D // P
    hi, lo = _hilo(w.astype(np.float32), s_w)

    def pk(a):
        return a.reshape(n_h, P, n_d2, P).transpose(2, 1, 0, 3)

    return np.ascontiguousarray(np.stack([pk(hi), pk(lo)], axis=2))


# ---------------------------------------------------------------- kernel
def _prepare(hidden_states, gate_w, e_bias, w_gate_up, w_down, ws_gate_up, ws_down):
    x = np.asarray(hidden_states, dtype=np.float32)
    topk_idx, topk_w = _route(x, np.asarray(gate_w), np.asarray(e_bias))

    # dispatch: token lists per expert, sorted-stable by expert id
    flat_e = topk_idx.ravel()
    order = np.argsort(flat_e, kind="stable")
    pair_tok = order // TOP_K
    counts = np.bincount(flat_e, minlength=E)
    starts = np.zeros(E + 1, np.int64)
    np.cumsum(counts, out=starts[1:])

    # expert -> core assignment: pair largest with smallest; overflow beyond
    # the slot caps is cut into per-core pieces handled by slot C
    by_count = np.argsort(-counts, kind="stable")
    slotA = by_count[:N_CORES]
    slotB = by_count[E - 1 : N_CORES - 1 : -1]  # reversed smallest half
    C1, C2, C3 = _solve_caps(counts)

    # overflow pieces: (expert, local_start, n_tokens), each <= C3
    pieces = []
    cap_of = {}
    for c in range(N_CORES):
        cap_of[int(slotA[c])] = C1
        cap_of[int(slotB[c])] = C2
    for e_id in range(E):
        ov = int(counts[e_id]) - cap_of[e_id]
        o = cap_of[e_id]
        while ov > 0:
            take = min(ov, C3)
            pieces.append((e_id, o, take))
            o += take
            ov -= take
    assert len(pieces) <= N_CORES, (pieces, C1, C2, C3)

    # global quantization scales -> SPMD-safe immediate dequant factors
    w_gate_up = np.asarray(w_gate_up, dtype=np.float32)
    w_down = np.asarray(w_down, dtype=np.float32)
    ws_gate_up = np.asarray(ws_gate_up, dtype=np.float32)
    ws_down = np.asarray(ws_down, dtype=np.float32)
    s_x = 224.0 / max(float(np.abs(x).max()), 1e-30)
    s_w1 = 224.0 / max(float(np.abs(w_gate_up).max()),
                       float(np.abs(ws_gate_up).max()), 1e-30)
    s_w2 = 224.0 / max(float(np.abs(w_down).max()),
                       float(np.abs(ws_down).max()), 1e-30)
    inv_g = 1.0 / (s_x * s_w1)          # dequant of gate_up psum
    c_u = S_H / (s_x * s_w1)            # dequant of u * h-quant scale
    c_y = 1.0 / (S_H * s_w2)            # dequant of down psum (shared)
    pair_w = (topk_w.ravel()[order] * np.float32(ROUTED_SCALE * c_y)).astype(
        np.float32
    )

    nc = _build_program(C1, C2, C3, inv_g, c_u, c_y)

    xT = np.ascontiguousarray(x.T)  # [D, T]
    xTh, xTl = _hilo(xT, s_x)

    w1_p = {int(e): _pack_w1(w_gate_up[e], 2 * I, s_w1) for e in range(E)}
    w2_p = {int(e): _pack_w2(w_down[e], I, s_w2) for e in range(E)}
    ws1_p = _pack_w1(ws_gate_up, 2 * SHARED_I, s_w1)
    ws2_p = _pack_w2(ws_down, SHARED_I, s_w2)

    def slot_inputs(m, e_id, lo, n_e, C, xh_name, xl_name, wr_name):
        idx = pair_tok[starts[e_id] + lo : starts[e_id] + lo + n_e]
        w = pair_w[starts[e_id] + lo : starts[e_id] + lo + n_e]
        idx_pad = np.zeros(C, np.int64)
        idx_pad[:n_e] = idx
        w_pad = np.zeros(C, np.float32)
        w_pad[:n_e] = w
        m[xh_name] = xTh[:, idx_pad]
        m[xl_name] = xTl[:, idx_pad]
        m[wr_name] = np.ascontiguousarray(np.broadcast_to(w_pad, (P, C)))
        return (idx, n_e)

    TS = T // N_CORES
    in_maps = []
    core_info = []
    for c in range(N_CORES):
        eA, eB = int(slotA[c]), int(slotB[c])
        m = {}
        info = []
        info.append(slot_inputs(m, eA, 0, min(int(counts[eA]), C1), C1,
                                "xt1h", "xt1l", "wr1"))
        info.append(slot_inputs(m, eB, 0, min(int(counts[eB]), C2), C2,
                                "xt2h", "xt2l", "wr2"))
        if c < len(pieces):
            eC, lo, n_c = pieces[c]
        else:
            eC, lo, n_c = eA, 0, 0   # idle slot C: wr=0 zeroes the output
        info.append(slot_inputs(m, eC, lo, n_c, C3, "xtch", "xtcl", "wrc"))
        m["xtsh"] = np.ascontiguousarray(xTh[:, c * TS : (c + 1) * TS])
        m["xtsl"] = np.ascontiguousarray(xTl[:, c * TS : (c + 1) * TS])
        m["w1a"] = w1_p[eA]
        m["w2a"] = w2_p[eA]
        m["w1b"] = w1_p[eB]
        m["w2b"] = w2_p[eB]
        m["w1c"] = w1_p[eC]
        m["w2c"] = w2_p[eC]
        m["ws1"] = ws1_p
        m["ws2"] = ws2_p
        in_maps.append(m)
        core_info.append(info)
    return nc, in_maps, core_info


def _combine(res_results, core_info):
    TS = T // N_CORES
    out = np.zeros((T, D), np.float32)
    for c in range(N_CORES):
        (idxA, nA), (idxB, nB), (idxC, nC) = core_info[c]
        out[idxA] += res_results[c]["y1"][:, :nA].T
        out[idxB] += res_results[c]["y2"][:, :nB].T
        if nC:
            out[idxC] += res_results[c]["yc"][:, :nC].T
        out[c * TS : (c + 1) * TS] += res_results[c]["ys"].T
    return out


def kernel(hidden_states, gate_w, e_bias, w_gate_up, w_down, ws_gate_up, ws_down):
    nc, in_maps, core_info = _prepare(
        hidden_states, gate_w, e_bias, w_gate_up, w_down, ws_gate_up, ws_down
    )
    res = run_bass_kernel_spmd(nc, in_maps, list(range(N_CORES)))
    return _combine(res.results, core_info)
